# revision 13
# baseline (speedup 1.0000x reference)
"""Self-contained 8-core Trainium2 Bass kernel for a 2-layer GCN
(PyG GCNConv semantics: add self-loops, symmetric normalization).

Strategy
--------
The symmetric edge norm factorizes: norm(s,d) = dinv[s]*dinv[d].  So each
layer is:
    h' = (x @ W) * dinv[node]            (dense matmul, node-major)
    agg[d] = sum_{e: dst=d} h'[src_e]    (gather + segment-sum)
    out[d] = dinv[d]*agg[d] + b          (scale + rank-1 bias)

Sharding: nodes are range-sharded across 8 cores (12500 each); edges are
partitioned by destination core.  Each core computes its h' shard, then the
shards are AllGathered so every core can gather arbitrary source rows.  The
gather uses the GPSIMD dma_gather instruction (int16 indices -> the gathered
table is split into 4 "banks" of <=32768 rows).  The segment-sum is done on
the TensorEngine: edges are sorted by destination, chunked 128 at a time, and
a one-hot selection matrix S[edge, dst_rel] (built with one DVE is_equal op
per chunk) turns the scatter-add into PSUM-accumulated matmuls per 128-dst
tile.  bf16 is used for the gathered values + S (PSUM still accumulates fp32).

The whole thing is one SPMD program: per-core variation lives entirely in the
input data (indices, dst_rel, per-core x slab).  Group sizes are padded to the
max over cores so the instruction stream is identical on every core.
"""

import math
import numpy as np
import ml_dtypes

import concourse.bacc as bacc
import concourse.bass as bass
import concourse.mybir as mybir
import concourse.tile as tile
from concourse import bass_utils

BF = ml_dtypes.bfloat16

# Problem constants (nn_GCN: x [100000,128], edge_index [2,1600000],
# W1 [128,128], b1 [128], W2 [128,40], b2 [40]).
N_NODES = 100000
N_CORES = 8
P = 128
BANKS = 4
F_IN = 128
F_HID = 128
F_OUT = 40
F_PAD = 128  # padded storage width for layer-2 rows (256B in bf16)


def _cdiv(a, b):
    return -(-a // b)


# ---------------------------------------------------------------------------
# Host-side preprocessing (pure numpy; index/structure work only)
# ---------------------------------------------------------------------------

def preprocess(edge_index, n, GB=1):
    C, B = N_CORES, BANKS
    per_core = n // C
    assert per_core * C == n and per_core % B == 0
    quarter = per_core // B
    bank_rows = quarter * C
    assert bank_rows < 32768
    T = _cdiv(per_core, P)
    last_rows = per_core - (T - 1) * P

    src = np.asarray(edge_index[0]).astype(np.int64)
    dst = np.asarray(edge_index[1]).astype(np.int64)
    loop = np.arange(n, dtype=np.int64)
    src_f = np.concatenate([src, loop])
    dst_f = np.concatenate([dst, loop])

    deg = np.bincount(dst_f, minlength=n).astype(np.float64)  # >= 1 (self loop)
    dinv = (1.0 / np.sqrt(deg)).astype(np.float32)
    sqdeg = np.sqrt(deg).astype(np.float32)

    core = dst_f // per_core
    dst_local = dst_f - core * per_core
    t_idx = dst_local // P
    rel = (dst_local - t_idx * P).astype(np.float32)
    bank = (src_f % per_core) // quarter
    row16 = (src_f // per_core) * quarter + (src_f % per_core) % quarter

    TB = T * B
    # group ordering: (block of GB tiles, bank, tile-in-block) so that one
    # dma_gather covers a whole (block, bank) stripe of contiguous slots
    NBLK = _cdiv(T, GB)
    blk = t_idx // GB
    tin = t_idx - blk * GB
    gorder = (blk * B + bank) * GB + tin          # position of group (t,b)
    # NOTE: trailing partial block simply has fewer tiles; gorder stays unique
    key = core * (NBLK * B * GB) + gorder
    NG = NBLK * B * GB
    counts_g = np.bincount(key, minlength=C * NG).reshape(C, NG)
    K_g = _cdiv_arr(counts_g.max(axis=0), P)       # [NG] chunks per group
    slots_g = K_g * P
    off_g = np.zeros(NG + 1, dtype=np.int64)
    np.cumsum(slots_g, out=off_g[1:])
    E_slots = int(off_g[-1])
    n_chunks = E_slots // P
    cols16 = E_slots // 16

    idx16 = np.zeros((C, E_slots), dtype=np.int16)   # pad -> row 0 (harmless)
    relpad = np.full((C, E_slots), -1.0, dtype=np.float32)  # pad -> no dst match

    order = np.argsort(key, kind="stable")
    key_s = key[order]
    row_s = row16[order]
    rel_s = rel[order]
    run_starts = np.searchsorted(key_s, np.arange(C * NG))
    pos = np.arange(len(key_s)) - run_starts[key_s]
    gcore = key_s // NG
    gkey = key_s % NG
    slot = off_g[gkey] + pos
    idx16[gcore, slot] = row_s.astype(np.int16)
    relpad[gcore, slot] = rel_s

    idx_packed = idx16.reshape(C, cols16, 16).transpose(0, 2, 1)  # [C,16,cols16]
    idx_packed = np.ascontiguousarray(np.tile(idx_packed, (1, 8, 1)))  # [C,128,cols16]
    rel_packed = np.ascontiguousarray(
        relpad.reshape(C, n_chunks, P).transpose(0, 2, 1)
    )  # [C,128,n_chunks] f32

    dv = np.zeros((C, T * P), np.float32)
    dv[:, :per_core] = dinv.reshape(C, per_core)
    dinv_part = np.ascontiguousarray(dv.reshape(C, T, P).transpose(0, 2, 1))  # [C,128,T]
    sq = np.zeros((C, 1, T * P), np.float32)
    sq[:, 0, :per_core] = sqdeg.reshape(C, per_core)
    sqdeg_free = sq  # [C,1,T*128] (single-partition row for rank-1 bias matmul)

    return dict(
        per_core=per_core, quarter=quarter, bank_rows=bank_rows, T=T,
        last_rows=last_rows, GB=GB, NBLK=NBLK, K_g=K_g, off_g=off_g,
        E_slots=E_slots, n_chunks=n_chunks, cols16=cols16,
        idx_packed=idx_packed, rel_packed=rel_packed, dinv_part=dinv_part,
        sqdeg_free=sqdeg_free,
    )


def _cdiv_arr(a, b):
    return -(-a // b)


# ---------------------------------------------------------------------------
# Program builder (one SPMD program for all 8 cores)
# ---------------------------------------------------------------------------

def build_program(meta, debug=False, stages=("A", "AG1", "B", "AG2", "C")):
    f32 = mybir.dt.float32
    bf = mybir.dt.bfloat16
    i16 = mybir.dt.int16
    Copy = mybir.ActivationFunctionType.Copy
    Relu = mybir.ActivationFunctionType.Relu

    T = meta["T"]
    B = BANKS
    GB = meta["GB"]
    NBLK = meta["NBLK"]
    K_g = meta["K_g"]
    off_g = meta["off_g"]
    per_core = meta["per_core"]
    quarter = meta["quarter"]
    bank_rows = meta["bank_rows"]
    n_chunks = meta["n_chunks"]
    cols16 = meta["cols16"]
    last_rows = meta["last_rows"]

    nc = bacc.Bacc(
        "TRN2", target_bir_lowering=False, debug=debug, num_devices=N_CORES
    )

    xT_d = nc.dram_tensor("xT", [F_IN, per_core], f32, kind="ExternalInput")
    w1_d = nc.dram_tensor("w1", [F_IN, F_HID], f32, kind="ExternalInput")
    w2_d = nc.dram_tensor("w2", [F_HID, F_PAD], f32, kind="ExternalInput")
    b1_d = nc.dram_tensor("b1", [1, F_HID], f32, kind="ExternalInput")
    b2_d = nc.dram_tensor("b2", [1, F_PAD], f32, kind="ExternalInput")
    idx_d = nc.dram_tensor("idx16", [P, cols16], i16, kind="ExternalInput")
    rel_d = nc.dram_tensor("rel", [P, n_chunks], f32, kind="ExternalInput")
    dinv_d = nc.dram_tensor("dinv_p", [P, T], f32, kind="ExternalInput")
    sqdeg_d = nc.dram_tensor("sqdeg_f", [1, T * P], f32, kind="ExternalInput")
    iota_d = nc.dram_tensor("iota_bf", [P, P], bf, kind="ExternalInput")
    ident_d = nc.dram_tensor("ident", [P, P], f32, kind="ExternalInput")
    hid_out = nc.dram_tensor("hidden_out", [per_core, F_HID], f32, kind="ExternalOutput")
    log_out = nc.dram_tensor("logits_out", [per_core, F_OUT], f32, kind="ExternalOutput")

    h1_shard = nc.dram_tensor("h1_shard", [per_core, F_HID], bf)
    h2_shard = nc.dram_tensor("h2_shard", [per_core, F_PAD], bf)
    T1 = [
        nc.dram_tensor(f"T1_{k}", [bank_rows, F_HID], bf, addr_space="Shared")
        for k in range(B)
    ]
    T2 = [
        nc.dram_tensor(f"T2_{k}", [bank_rows, F_PAD], bf, addr_space="Shared")
        for k in range(B)
    ]
    rg = [list(range(N_CORES))]

    with tile.TileContext(nc) as tc:
        with (
            tc.tile_pool(name="const", bufs=1) as cp,
            tc.tile_pool(name="io", bufs=6) as iop,
            tc.tile_pool(name="s", bufs=16) as sp,
            tc.tile_pool(name="g", bufs=10) as gp,
            tc.tile_pool(name="agg", bufs=4, space="PSUM") as aggp,
            tc.tile_pool(name="tp", bufs=4, space="PSUM") as tpp,
        ):
            # ---- load constants / resident data
            w1_sb = cp.tile([F_IN, F_HID], f32)
            nc.sync.dma_start(w1_sb[:], w1_d[:, :])
            w2_sb = cp.tile([F_HID, F_PAD], f32)
            nc.sync.dma_start(w2_sb[:], w2_d[:, :])
            b1_sb = cp.tile([1, F_HID], f32)
            nc.sync.dma_start(b1_sb[:], b1_d[:, :])
            b2_sb = cp.tile([1, F_PAD], f32)
            nc.sync.dma_start(b2_sb[:], b2_d[:, :])
            iota_sb = cp.tile([P, P], bf)
            nc.sync.dma_start(iota_sb[:], iota_d[:, :])
            ident_sb = cp.tile([P, P], f32)
            nc.sync.dma_start(ident_sb[:], ident_d[:, :])
            dinv_sb = cp.tile([P, T], f32)
            nc.sync.dma_start(dinv_sb[:], dinv_d[:, :])
            sqdeg_sb = cp.tile([1, T * P], f32)
            nc.sync.dma_start(sqdeg_sb[:], sqdeg_d[:, :])
            idx_sb = cp.tile([P, cols16], i16)
            nc.sync.dma_start(idx_sb[:], idx_d[:, :])
            rel_sb = cp.tile([P, n_chunks], f32)
            nc.sync.dma_start(rel_sb[:], rel_d[:, :])

            # ---- phase A: h1' = (x @ W1) * dinv  (local shard, bf16)
            for t in range(T if "A" in stages else 0):
                rows = P if t < T - 1 else last_rows
                lo = t * P
                xT_t = iop.tile([P, P], f32, tag="xT_t")
                nc.sync.dma_start(xT_t[:, :rows], xT_d[:, lo : lo + rows])
                hps = tpp.tile([P, F_HID], f32, tag="tp")
                nc.tensor.matmul(
                    out=hps[:rows, :], lhsT=xT_t[:, :rows], rhs=w1_sb[:],
                    start=True, stop=True,
                )
                hp = iop.tile([P, F_HID], bf, tag="hp")
                nc.scalar.activation(
                    hp[:rows, :], hps[:rows, :], Copy, scale=dinv_sb[:rows, t : t + 1]
                )
                nc.sync.dma_start(h1_shard[lo : lo + rows, :], hp[:rows, :])

            # ---- AllGather h1' into 4 bank tables
            for k in range(B if "AG1" in stages else 0):
                nc.gpsimd.collective_compute(
                    "AllGather", mybir.AluOpType.bypass, replica_groups=rg,
                    ins=[h1_shard[k * quarter : (k + 1) * quarter, :]],
                    outs=[T1[k][:, :]],
                )

            # ---- edge pass helper: per (block of GB tiles, bank) one gather,
            # matmul segment-sum into one psum bank holding GB tile slices
            def edge_pass(banks, bias_sb, gtag, out_cb):
                for blk in range(NBLK):
                    tiles = list(range(blk * GB, min((blk + 1) * GB, T)))
                    nt = len(tiles)
                    GMAX = 6  # <=768 idxs per dma_gather (SWDGE ring capacity)
                    gtiles = []
                    for b in range(B):
                        g0 = (blk * B + b) * GB
                        Ktot = sum(int(K_g[g0 + i]) for i in range(nt))
                        if Ktot == 0:
                            continue
                        off = int(off_g[g0])
                        parts = []
                        done = 0
                        while done < Ktot:
                            kk = min(GMAX, Ktot - done)
                            gsub = gp.tile([P, kk, F_PAD], bf, tag=gtag)
                            o = off + done * P
                            nc.gpsimd.dma_gather(
                                gsub[:, :, :], banks[b][:, :],
                                idx_sb[:, o // 16 : (o + kk * P) // 16],
                                kk * P, kk * P, F_PAD,
                            )
                            parts.append((done, kk, gsub))
                            done += kk

                        def mk_sel(parts):
                            def sel(cc):
                                for st, kk, gsub in parts:
                                    if cc < st + kk:
                                        return gsub[:, cc - st, :]
                                raise IndexError(cc)
                            return sel
                        gtiles.append((mk_sel(parts), g0))
                    for i, t in enumerate(tiles):
                        ps = aggp.tile([P, F_PAD], f32, tag="agg")
                        started = False
                        for sel, g0 in gtiles:
                            base = int(off_g[g0])
                            coff = (int(off_g[g0 + i]) - base) // P
                            for c in range(int(K_g[g0 + i])):
                                ch = int(off_g[g0 + i]) // P + c
                                S = sp.tile([P, P], bf, tag="S")
                                nc.vector.tensor_scalar(
                                    S[:], iota_sb[:], rel_sb[:, ch : ch + 1], None,
                                    mybir.AluOpType.is_equal,
                                )
                                nc.tensor.matmul(
                                    out=ps[:], lhsT=S[:], rhs=sel(coff + c),
                                    start=not started, stop=False,
                                )
                                started = True
                        rows = P if t < T - 1 else last_rows
                        nc.tensor.matmul(
                            out=ps[:],
                            lhsT=sqdeg_sb[0:1, t * P : (t + 1) * P],
                            rhs=bias_sb[0:1, :],
                            start=not started, stop=True,
                        )
                        out_cb(t, rows, ps[:])

            # ---- layer 1 epilogue per dst tile
            def l1_out(t, rows, ps):
                lo = t * P
                hid = iop.tile([P, F_HID], f32, tag="hid")
                nc.scalar.activation(hid[:], ps, Relu, scale=dinv_sb[:, t : t + 1])
                nc.sync.dma_start(hid_out[lo : lo + rows, :], hid[:rows, :])
                # layer-2 prep: h2' = (hidden @ W2) * dinv  (needs hidden^T)
                hTp = tpp.tile([P, P], f32, tag="tp")
                nc.tensor.transpose(out=hTp[:], in_=hid[:], identity=ident_sb[:])
                hT = iop.tile([P, P], f32, tag="hT")
                nc.scalar.copy(hT[:], hTp[:])
                h2ps = tpp.tile([P, F_PAD], f32, tag="tp")
                nc.tensor.matmul(
                    out=h2ps[:], lhsT=hT[:], rhs=w2_sb[:], start=True, stop=True
                )
                h2p = iop.tile([P, F_PAD], bf, tag="h2p")
                nc.scalar.activation(h2p[:], h2ps[:], Copy, scale=dinv_sb[:, t : t + 1])
                nc.sync.dma_start(h2_shard[lo : lo + rows, :], h2p[:rows, :])

            def l2_out(t, rows, ps):
                lo = t * P
                lg = iop.tile([P, F_PAD], f32, tag="lg")
                nc.scalar.activation(lg[:], ps, Copy, scale=dinv_sb[:, t : t + 1])
                nc.sync.dma_start(log_out[lo : lo + rows, :], lg[:rows, :F_OUT])

            if "B" in stages:
                edge_pass(T1, b1_sb, "gA", l1_out)

            for k in range(B if "AG2" in stages else 0):
                nc.gpsimd.collective_compute(
                    "AllGather", mybir.AluOpType.bypass, replica_groups=rg,
                    ins=[h2_shard[k * quarter : (k + 1) * quarter, :]],
                    outs=[T2[k][:, :]],
                )

            if "C" in stages:
                edge_pass(T2, b2_sb, "gB", l2_out)

    nc.compile()
    return nc


# ---------------------------------------------------------------------------
# Input assembly + runner
# ---------------------------------------------------------------------------

def make_in_maps(x, W1, b1, W2, b2, meta):
    C = N_CORES
    per_core = meta["per_core"]
    x = np.asarray(x, dtype=np.float32)
    W1 = np.asarray(W1, dtype=np.float32)
    W2 = np.asarray(W2, dtype=np.float32)
    b1 = np.asarray(b1, dtype=np.float32)
    b2 = np.asarray(b2, dtype=np.float32)

    w2p = np.zeros((F_HID, F_PAD), np.float32)
    w2p[:, :F_OUT] = W2
    b2p = np.zeros((1, F_PAD), np.float32)
    b2p[0, :F_OUT] = b2
    iota_bf = np.broadcast_to(np.arange(P, dtype=np.float32), (P, P)).astype(BF)
    ident = np.eye(P, dtype=np.float32)

    in_maps = []
    for c in range(C):
        xT = np.ascontiguousarray(x[c * per_core : (c + 1) * per_core].T)
        in_maps.append(
            {
                "xT": xT,
                "w1": W1,
                "w2": w2p,
                "b1": b1.reshape(1, F_HID),
                "b2": b2p,
                "idx16": meta["idx_packed"][c],
                "rel": meta["rel_packed"][c],
                "dinv_p": meta["dinv_part"][c],
                "sqdeg_f": meta["sqdeg_free"][c],
                "iota_bf": np.ascontiguousarray(iota_bf),
                "ident": ident,
            }
        )
    return in_maps


def run_gcn(x, edge_index, W1, b1, W2, b2, trace=False, trace_cores=None):
    n = x.shape[0]
    meta = preprocess(edge_index, n)
    nc = build_program(meta, debug=False)
    in_maps = make_in_maps(x, W1, b1, W2, b2, meta)
    res = bass_utils.run_bass_kernel_spmd(
        nc, in_maps, core_ids=list(range(N_CORES)), trace=trace,
        trace_cores=trace_cores,
    )
    per_core = meta["per_core"]
    hidden = np.concatenate([res.results[c]["hidden_out"] for c in range(N_CORES)], axis=0)
    logits = np.concatenate([res.results[c]["logits_out"] for c in range(N_CORES)], axis=0)
    return (logits, hidden), res


def kernel(x, edge_index, W1, b1, W2, b2):
    (logits, hidden), _ = run_gcn(x, edge_index, W1, b1, W2, b2)
    return (
        np.asarray(logits, dtype=np.float32),
        np.asarray(hidden, dtype=np.float32),
    )


# revision 14
# speedup vs baseline: 2.0988x; 2.0988x over previous
"""Self-contained 8-core Trainium2 Bass kernel for a 2-layer GCN
(PyG GCNConv semantics: add self-loops, symmetric normalization).

Strategy
--------
The symmetric edge norm factorizes: norm(s,d) = dinv[s]*dinv[d].  So each
layer is:
    h' = (x @ W) * dinv[node]            (dense matmul, node-major)
    agg[d] = sum_{e: dst=d} h'[src_e]    (gather + segment-sum)
    out[d] = dinv[d]*agg[d] + b          (scale + rank-1 bias)

Sharding: nodes are range-sharded across 8 cores (12500 each); edges are
partitioned by destination core.  Each core computes its h' shard, then the
shards are AllGathered so every core can gather arbitrary source rows.  The
gather uses the GPSIMD dma_gather instruction (int16 indices -> the gathered
table is split into 4 "banks" of <=32768 rows).  The segment-sum is done on
the TensorEngine: edges are sorted by destination, chunked 128 at a time, and
a one-hot selection matrix S[edge, dst_rel] (built with one DVE is_equal op
per chunk) turns the scatter-add into PSUM-accumulated matmuls per 128-dst
tile.  bf16 is used for the gathered values + S (PSUM still accumulates fp32).

The whole thing is one SPMD program: per-core variation lives entirely in the
input data (indices, dst_rel, per-core x slab).  Group sizes are padded to the
max over cores so the instruction stream is identical on every core.
"""

import math
import numpy as np
import ml_dtypes

import concourse.bacc as bacc
import concourse.bass as bass
import concourse.mybir as mybir
import concourse.tile as tile
from concourse import bass_utils

BF = ml_dtypes.bfloat16

# Problem constants (nn_GCN: x [100000,128], edge_index [2,1600000],
# W1 [128,128], b1 [128], W2 [128,40], b2 [40]).
N_NODES = 100000
N_CORES = 8
P = 128
BANKS = 4
F_IN = 128
F_HID = 128
F_OUT = 40
F_PAD = 128  # padded storage width for layer-2 rows (256B in bf16)


def _cdiv(a, b):
    return -(-a // b)


# ---------------------------------------------------------------------------
# Host-side preprocessing (pure numpy; index/structure work only)
# ---------------------------------------------------------------------------

def preprocess(edge_index, n, GB=1):
    C, B = N_CORES, BANKS
    per_core = n // C
    assert per_core * C == n and per_core % B == 0
    quarter = per_core // B
    bank_rows = quarter * C
    assert bank_rows < 32768
    T = _cdiv(per_core, P)
    last_rows = per_core - (T - 1) * P

    src = np.asarray(edge_index[0]).astype(np.int64)
    dst = np.asarray(edge_index[1]).astype(np.int64)
    loop = np.arange(n, dtype=np.int64)
    src_f = np.concatenate([src, loop])
    dst_f = np.concatenate([dst, loop])

    deg = np.bincount(dst_f, minlength=n).astype(np.float64)  # >= 1 (self loop)
    dinv = (1.0 / np.sqrt(deg)).astype(np.float32)
    sqdeg = np.sqrt(deg).astype(np.float32)

    core = dst_f // per_core
    dst_local = dst_f - core * per_core
    t_idx = dst_local // P
    rel = (dst_local - t_idx * P).astype(np.float32)
    bank = (src_f % per_core) // quarter
    row16 = (src_f // per_core) * quarter + (src_f % per_core) % quarter

    TB = T * B
    # group ordering: (block of GB tiles, bank, tile-in-block) so that one
    # dma_gather covers a whole (block, bank) stripe of contiguous slots
    NBLK = _cdiv(T, GB)
    blk = t_idx // GB
    tin = t_idx - blk * GB
    gorder = (blk * B + bank) * GB + tin          # position of group (t,b)
    # NOTE: trailing partial block simply has fewer tiles; gorder stays unique
    key = core * (NBLK * B * GB) + gorder
    NG = NBLK * B * GB
    counts_g = np.bincount(key, minlength=C * NG).reshape(C, NG)
    K_g = _cdiv_arr(counts_g.max(axis=0), P)       # [NG] chunks per group
    slots_g = K_g * P
    off_g = np.zeros(NG + 1, dtype=np.int64)
    np.cumsum(slots_g, out=off_g[1:])
    E_slots = int(off_g[-1])
    n_chunks = E_slots // P
    cols16 = E_slots // 16

    idx16 = np.zeros((C, E_slots), dtype=np.int16)   # pad -> row 0 (harmless)
    relpad = np.full((C, E_slots), -1.0, dtype=np.float32)  # pad -> no dst match

    order = np.argsort(key, kind="stable")
    key_s = key[order]
    row_s = row16[order]
    rel_s = rel[order]
    run_starts = np.searchsorted(key_s, np.arange(C * NG))
    pos = np.arange(len(key_s)) - run_starts[key_s]
    gcore = key_s // NG
    gkey = key_s % NG
    slot = off_g[gkey] + pos
    idx16[gcore, slot] = row_s.astype(np.int16)
    relpad[gcore, slot] = rel_s

    idx_packed = idx16.reshape(C, cols16, 16).transpose(0, 2, 1)  # [C,16,cols16]
    idx_packed = np.ascontiguousarray(np.tile(idx_packed, (1, 8, 1)))  # [C,128,cols16]
    rel_packed = np.ascontiguousarray(
        relpad.reshape(C, n_chunks, P).transpose(0, 2, 1)
    )  # [C,128,n_chunks] f32

    dv = np.zeros((C, T * P), np.float32)
    dv[:, :per_core] = dinv.reshape(C, per_core)
    dinv_part = np.ascontiguousarray(dv.reshape(C, T, P).transpose(0, 2, 1))  # [C,128,T]
    sq = np.zeros((C, 1, T * P), np.float32)
    sq[:, 0, :per_core] = sqdeg.reshape(C, per_core)
    sqdeg_free = sq  # [C,1,T*128] (single-partition row for rank-1 bias matmul)

    return dict(
        per_core=per_core, quarter=quarter, bank_rows=bank_rows, T=T,
        last_rows=last_rows, GB=GB, NBLK=NBLK, K_g=K_g, off_g=off_g,
        E_slots=E_slots, n_chunks=n_chunks, cols16=cols16,
        idx_packed=idx_packed, rel_packed=rel_packed, dinv_part=dinv_part,
        sqdeg_free=sqdeg_free,
    )


def _cdiv_arr(a, b):
    return -(-a // b)


# ---------------------------------------------------------------------------
# Program builder (one SPMD program for all 8 cores)
# ---------------------------------------------------------------------------

def build_program(meta, debug=False, stages=("A", "AG1", "B", "AG2", "C")):
    f32 = mybir.dt.float32
    bf = mybir.dt.bfloat16
    i16 = mybir.dt.int16
    Copy = mybir.ActivationFunctionType.Copy
    Relu = mybir.ActivationFunctionType.Relu

    T = meta["T"]
    B = BANKS
    GB = meta["GB"]
    NBLK = meta["NBLK"]
    K_g = meta["K_g"]
    off_g = meta["off_g"]
    per_core = meta["per_core"]
    quarter = meta["quarter"]
    bank_rows = meta["bank_rows"]
    n_chunks = meta["n_chunks"]
    cols16 = meta["cols16"]
    last_rows = meta["last_rows"]

    nc = bacc.Bacc(
        "TRN2", target_bir_lowering=False, debug=debug, num_devices=N_CORES
    )

    xT_d = nc.dram_tensor("xT", [F_IN, per_core], f32, kind="ExternalInput")
    w1_d = nc.dram_tensor("w1", [F_IN, F_HID], f32, kind="ExternalInput")
    w2_d = nc.dram_tensor("w2", [F_HID, F_PAD], f32, kind="ExternalInput")
    b1_d = nc.dram_tensor("b1", [1, F_HID], f32, kind="ExternalInput")
    b2_d = nc.dram_tensor("b2", [1, F_PAD], f32, kind="ExternalInput")
    idx_d = nc.dram_tensor("idx16", [P, cols16], i16, kind="ExternalInput")
    rel_d = nc.dram_tensor("rel", [P, n_chunks], f32, kind="ExternalInput")
    dinv_d = nc.dram_tensor("dinv_p", [P, T], f32, kind="ExternalInput")
    sqdeg_d = nc.dram_tensor("sqdeg_f", [1, T * P], f32, kind="ExternalInput")
    iota_d = nc.dram_tensor("iota_bf", [P, P], bf, kind="ExternalInput")
    ident_d = nc.dram_tensor("ident", [P, P], f32, kind="ExternalInput")
    hid_out = nc.dram_tensor("hidden_out", [per_core, F_HID], f32, kind="ExternalOutput")
    log_out = nc.dram_tensor("logits_out", [per_core, F_OUT], f32, kind="ExternalOutput")

    h1_shard = nc.dram_tensor("h1_shard", [per_core, F_HID], bf)
    h2_shard = nc.dram_tensor("h2_shard", [per_core, F_PAD], bf)
    T1 = [
        nc.dram_tensor(f"T1_{k}", [bank_rows, F_HID], bf, addr_space="Shared")
        for k in range(B)
    ]
    T2 = [
        nc.dram_tensor(f"T2_{k}", [bank_rows, F_PAD], bf, addr_space="Shared")
        for k in range(B)
    ]
    rg = [list(range(N_CORES))]

    with tile.TileContext(nc) as tc:
        with (
            tc.tile_pool(name="const", bufs=1) as cp,
            tc.tile_pool(name="io", bufs=6) as iop,
            tc.tile_pool(name="s", bufs=16) as sp,
            tc.tile_pool(name="g", bufs=10) as gp,
            tc.tile_pool(name="agg", bufs=4, space="PSUM") as aggp,
            tc.tile_pool(name="tp", bufs=4, space="PSUM") as tpp,
        ):
            # ---- load constants / resident data
            w1_sb = cp.tile([F_IN, F_HID], f32)
            nc.sync.dma_start(w1_sb[:], w1_d[:, :])
            w2_sb = cp.tile([F_HID, F_PAD], f32)
            nc.sync.dma_start(w2_sb[:], w2_d[:, :])
            b1_sb = cp.tile([1, F_HID], f32)
            nc.sync.dma_start(b1_sb[:], b1_d[:, :])
            b2_sb = cp.tile([1, F_PAD], f32)
            nc.sync.dma_start(b2_sb[:], b2_d[:, :])
            iota_sb = cp.tile([P, P], bf)
            nc.sync.dma_start(iota_sb[:], iota_d[:, :])
            ident_sb = cp.tile([P, P], f32)
            nc.sync.dma_start(ident_sb[:], ident_d[:, :])
            dinv_sb = cp.tile([P, T], f32)
            nc.sync.dma_start(dinv_sb[:], dinv_d[:, :])
            sqdeg_sb = cp.tile([1, T * P], f32)
            nc.sync.dma_start(sqdeg_sb[:], sqdeg_d[:, :])
            idx_sb = cp.tile([P, cols16], i16)
            nc.sync.dma_start(idx_sb[:], idx_d[:, :])
            rel_sb = cp.tile([P, n_chunks], f32)
            nc.sync.dma_start(rel_sb[:], rel_d[:, :])

            # ---- phase A: h1' = (x @ W1) * dinv  (local shard, bf16)
            for t in range(T if "A" in stages else 0):
                rows = P if t < T - 1 else last_rows
                lo = t * P
                xT_t = iop.tile([P, P], f32, tag="xT_t")
                nc.sync.dma_start(xT_t[:, :rows], xT_d[:, lo : lo + rows])
                hps = tpp.tile([P, F_HID], f32, tag="tp")
                nc.tensor.matmul(
                    out=hps[:rows, :], lhsT=xT_t[:, :rows], rhs=w1_sb[:],
                    start=True, stop=True,
                )
                hp = iop.tile([P, F_HID], bf, tag="hp")
                nc.scalar.activation(
                    hp[:rows, :], hps[:rows, :], Copy, scale=dinv_sb[:rows, t : t + 1]
                )
                nc.sync.dma_start(h1_shard[lo : lo + rows, :], hp[:rows, :])

            # ---- AllGather h1' into 4 bank tables
            for k in range(B if "AG1" in stages else 0):
                nc.gpsimd.collective_compute(
                    "AllGather", mybir.AluOpType.bypass, replica_groups=rg,
                    ins=[h1_shard[k * quarter : (k + 1) * quarter, :]],
                    outs=[T1[k][:, :]],
                )

            # ---- edge pass helper: per (block of GB tiles, bank) one gather,
            # matmul segment-sum into one psum bank holding GB tile slices
            def edge_pass(banks, bias_sb, gtag, out_cb):
                for blk in range(NBLK):
                    tiles = list(range(blk * GB, min((blk + 1) * GB, T)))
                    nt = len(tiles)
                    GMAX = 7  # 896 idxs = 58 descs/engine, inside the 64-desc SWDGE ring
                    gtiles = []
                    for b in range(B):
                        g0 = (blk * B + b) * GB
                        Ktot = sum(int(K_g[g0 + i]) for i in range(nt))
                        if Ktot == 0:
                            continue
                        off = int(off_g[g0])
                        parts = []
                        done = 0
                        while done < Ktot:
                            kk = min(GMAX, Ktot - done)
                            gsub = gp.tile([P, kk, F_PAD], bf, tag=gtag)
                            o = off + done * P
                            nc.gpsimd.dma_gather(
                                gsub[:, :, :], banks[b][:, :],
                                idx_sb[:, o // 16 : (o + kk * P) // 16],
                                kk * P, kk * P, F_PAD,
                            )
                            parts.append((done, kk, gsub))
                            done += kk

                        def mk_sel(parts):
                            def sel(cc):
                                for st, kk, gsub in parts:
                                    if cc < st + kk:
                                        return gsub[:, cc - st, :]
                                raise IndexError(cc)
                            return sel
                        gtiles.append((mk_sel(parts), g0))
                    for i, t in enumerate(tiles):
                        ps = aggp.tile([P, F_PAD], f32, tag="agg")
                        started = False
                        for sel, g0 in gtiles:
                            base = int(off_g[g0])
                            coff = (int(off_g[g0 + i]) - base) // P
                            for c in range(int(K_g[g0 + i])):
                                ch = int(off_g[g0 + i]) // P + c
                                S = sp.tile([P, P], bf, tag="S")
                                nc.vector.tensor_scalar(
                                    S[:], iota_sb[:], rel_sb[:, ch : ch + 1], None,
                                    mybir.AluOpType.is_equal,
                                )
                                nc.tensor.matmul(
                                    out=ps[:], lhsT=S[:], rhs=sel(coff + c),
                                    start=not started, stop=False,
                                )
                                started = True
                        rows = P if t < T - 1 else last_rows
                        nc.tensor.matmul(
                            out=ps[:],
                            lhsT=sqdeg_sb[0:1, t * P : (t + 1) * P],
                            rhs=bias_sb[0:1, :],
                            start=not started, stop=True,
                        )
                        out_cb(t, rows, ps[:])

            # ---- layer 1 epilogue per dst tile
            def l1_out(t, rows, ps):
                lo = t * P
                hid = iop.tile([P, F_HID], f32, tag="hid")
                nc.scalar.activation(hid[:], ps, Relu, scale=dinv_sb[:, t : t + 1])
                nc.sync.dma_start(hid_out[lo : lo + rows, :], hid[:rows, :])
                # layer-2 prep: h2' = (hidden @ W2) * dinv  (needs hidden^T)
                hTp = tpp.tile([P, P], f32, tag="tp")
                nc.tensor.transpose(out=hTp[:], in_=hid[:], identity=ident_sb[:])
                hT = iop.tile([P, P], f32, tag="hT")
                nc.scalar.copy(hT[:], hTp[:])
                h2ps = tpp.tile([P, F_PAD], f32, tag="tp")
                nc.tensor.matmul(
                    out=h2ps[:], lhsT=hT[:], rhs=w2_sb[:], start=True, stop=True
                )
                h2p = iop.tile([P, F_PAD], bf, tag="h2p")
                nc.scalar.activation(h2p[:], h2ps[:], Copy, scale=dinv_sb[:, t : t + 1])
                nc.sync.dma_start(h2_shard[lo : lo + rows, :], h2p[:rows, :])

            def l2_out(t, rows, ps):
                lo = t * P
                lg = iop.tile([P, F_PAD], f32, tag="lg")
                nc.scalar.activation(lg[:], ps, Copy, scale=dinv_sb[:, t : t + 1])
                nc.sync.dma_start(log_out[lo : lo + rows, :], lg[:rows, :F_OUT])

            if "B" in stages:
                edge_pass(T1, b1_sb, "gA", l1_out)

            for k in range(B if "AG2" in stages else 0):
                nc.gpsimd.collective_compute(
                    "AllGather", mybir.AluOpType.bypass, replica_groups=rg,
                    ins=[h2_shard[k * quarter : (k + 1) * quarter, :]],
                    outs=[T2[k][:, :]],
                )

            if "C" in stages:
                edge_pass(T2, b2_sb, "gB", l2_out)

    nc.compile()
    return nc


# ---------------------------------------------------------------------------
# Input assembly + runner
# ---------------------------------------------------------------------------

def make_in_maps(x, W1, b1, W2, b2, meta):
    C = N_CORES
    per_core = meta["per_core"]
    x = np.asarray(x, dtype=np.float32)
    W1 = np.asarray(W1, dtype=np.float32)
    W2 = np.asarray(W2, dtype=np.float32)
    b1 = np.asarray(b1, dtype=np.float32)
    b2 = np.asarray(b2, dtype=np.float32)

    w2p = np.zeros((F_HID, F_PAD), np.float32)
    w2p[:, :F_OUT] = W2
    b2p = np.zeros((1, F_PAD), np.float32)
    b2p[0, :F_OUT] = b2
    iota_bf = np.broadcast_to(np.arange(P, dtype=np.float32), (P, P)).astype(BF)
    ident = np.eye(P, dtype=np.float32)

    in_maps = []
    for c in range(C):
        xT = np.ascontiguousarray(x[c * per_core : (c + 1) * per_core].T)
        in_maps.append(
            {
                "xT": xT,
                "w1": W1,
                "w2": w2p,
                "b1": b1.reshape(1, F_HID),
                "b2": b2p,
                "idx16": meta["idx_packed"][c],
                "rel": meta["rel_packed"][c],
                "dinv_p": meta["dinv_part"][c],
                "sqdeg_f": meta["sqdeg_free"][c],
                "iota_bf": np.ascontiguousarray(iota_bf),
                "ident": ident,
            }
        )
    return in_maps


def run_gcn(x, edge_index, W1, b1, W2, b2, trace=False, trace_cores=None):
    n = x.shape[0]
    meta = preprocess(edge_index, n)
    nc = build_program(meta, debug=False)
    in_maps = make_in_maps(x, W1, b1, W2, b2, meta)
    res = bass_utils.run_bass_kernel_spmd(
        nc, in_maps, core_ids=list(range(N_CORES)), trace=trace,
        trace_cores=trace_cores,
    )
    per_core = meta["per_core"]
    hidden = np.concatenate([res.results[c]["hidden_out"] for c in range(N_CORES)], axis=0)
    logits = np.concatenate([res.results[c]["logits_out"] for c in range(N_CORES)], axis=0)
    return (logits, hidden), res


def kernel(x, edge_index, W1, b1, W2, b2):
    (logits, hidden), _ = run_gcn(x, edge_index, W1, b1, W2, b2)
    return (
        np.asarray(logits, dtype=np.float32),
        np.asarray(hidden, dtype=np.float32),
    )
